# revision 55
# baseline (speedup 1.0000x reference)
"""Trainium2 Bass kernel for nn_Attention_386547057357 (Transformer-XL style
relative-position sparse attention).

Sharding: data-parallel over batch — core c computes batch element c.
All weights replicated per core.

Math (per batch element):
    X = [memory; x]  (1024, 512)
    q = x @ W_q  (256, 512);  k = X @ W_k;  v = X @ W_v
    qhat = q + u_emb (per head);  qtld = q + v_emb
    ac[n, m]  = qhat_h[n] . k_h[m]                 (= term_a + term_c)
    bd[n, m]  = qtld_h[n] . RW[768 + n - m]        (= term_b + term_d)
    scores    = (ac + bd) * scale  + causal mask (m <= 768 + n)
    out = softmax(scores) @ v @ W_out + b_out

Key trick (rank-44 shift elimination): RW = R @ W_rel has rank <= 22 and
R's rows are sinusoids, so by the angle-addition identities

    bd_h[n, m] = sum_o  z_s,o[n] sin(w_o (768 + n - m))
                      + z_c,o[n] cos(w_o (768 + n - m))
               = sum_{k<44} u_k[n] * G_k[m]

with z_h = qtld_h @ W_rel_h^T (a K=64 -> 44 matmul),
u = [z_s*cosN, z_s*sinN, z_c*cosN, -z_c*sinN] (elementwise n-trig products)
and G = [sinM, cosM, cosM, sinM] host constants, sinM_o[m]=sin(w_o(768-m)).
So the relative-shift term is a plain K=44 matmul accumulated onto ac in
PSUM — no DRAM round trip, no shift at all.  The causal mask is a 0/1
bf16 host tile applied by the fused product+rowsum (scalar_tensor_tensor)
that also computes the softmax denominator.  Softmax skips max-subtraction
(logits are small; fp32 exp).  attn transpose for PV via gpsimd dma_gather.
All PE operands bf16; PSUM accumulation fp32; output fp32.

HW pitfall encoded here: all matmuls of one PSUM accumulation group must
share the lhsT/rhs base partition (PE tile row) — mixing base 0 and 64 in
one group hard-faults the device.  Hence the dual-offset copies of the
trig/G tables and per-head u rows at the head's own partition base.
"""

import sys

sys.path.insert(0, "/opt/trn_rl_repo")

import numpy as np
import ml_dtypes

import concourse.bass as bass
import concourse.mybir as mybir
import concourse.tile as tile
from concourse import bacc, library_config
from concourse.bass_utils import run_bass_kernel_spmd
from concourse.tile_rust import add_dep_helper

BF16 = ml_dtypes.bfloat16
F32 = np.float32

DIM = 512
NHEAD = 8
DHEAD = 64
CTX = 1024
NOCT = 11
B = 8
SEQ = 256
MEM = 768
TOT = MEM + SEQ  # 1024
SCALE = DHEAD ** -0.5  # 0.125
KEXP = 4 * NOCT  # 44: rank of the bd expansion

dt = mybir.dt
AF = mybir.ActivationFunctionType
ALU = mybir.AluOpType


# ---------------------------------------------------------------- bass program
def build_program():
    nc = bacc.Bacc("TRN2", target_bir_lowering=False, debug=False)

    xt_d = nc.dram_tensor("xt", [128, 4, TOT], dt.bfloat16, kind="ExternalInput")
    wqkv_d = nc.dram_tensor("wqkv", [128, 4, 1536], dt.bfloat16, kind="ExternalInput")
    wext_d = nc.dram_tensor("wext", [128, 4, KEXP], dt.bfloat16, kind="ExternalInput")
    trign_d = nc.dram_tensor("trign", [128, SEQ], dt.float32, kind="ExternalInput")
    gmat_d = nc.dram_tensor("gmat", [128, TOT], dt.bfloat16, kind="ExternalInput")
    mask_d = nc.dram_tensor("mask01", [128, 2, TOT], dt.bfloat16, kind="ExternalInput")
    wout_d = nc.dram_tensor("wout", [128, 4, 512], dt.bfloat16, kind="ExternalInput")
    bout_d = nc.dram_tensor("bout", [128, 512], dt.float32, kind="ExternalInput")
    u2_d = nc.dram_tensor("u2", [128, 1], dt.float32, kind="ExternalInput")
    v2_d = nc.dram_tensor("v2", [128, 1], dt.float32, kind="ExternalInput")
    gidx_d = nc.dram_tensor("gidx", [128, 16], dt.int16, kind="ExternalInput")
    out_d = nc.dram_tensor("out", [SEQ, 512], dt.float32, kind="ExternalOutput")

    with tile.TileContext(nc) as tc:
        _body(tc, xt_d, wqkv_d, wext_d, trign_d, gmat_d, mask_d, wout_d,
              bout_d, u2_d, v2_d, gidx_d, out_d)
    nc.compile()
    return nc


def _body(tc, xt_d, wqkv_d, wext_d, trign_d, gmat_d, mask_d, wout_d, bout_d,
          u2_d, v2_d, gidx_d, out_d):
    nc = tc.nc
    from contextlib import ExitStack

    with ExitStack() as ctx:
        consts = ctx.enter_context(tc.tile_pool(name="consts", bufs=1))

        # ---- loads.  The q projection only needs the x-part of X^T
        # (cols MEM:) and the q-columns of W_qkv — load those first so the
        # PE starts ~5us earlier; the k/v parts stream in behind.
        xt = consts.tile([128, 4, TOT], dt.bfloat16)
        wqkv = consts.tile([128, 4, 1536], dt.bfloat16)
        nc.sync.dma_start(xt[:, :, MEM:TOT], xt_d.ap()[:, :, MEM:TOT])
        nc.sync.dma_start(wqkv[:, :, 0:512], wqkv_d.ap()[:, :, 0:512])
        u2 = consts.tile([128, 1], dt.float32)
        nc.sync.dma_start(u2[:], u2_d.ap())
        v2 = consts.tile([128, 1], dt.float32)
        nc.sync.dma_start(v2[:], v2_d.ap())
        nc.sync.dma_start(wqkv[:, :, 512:1536], wqkv_d.ap()[:, :, 512:1536])
        nc.sync.dma_start(xt[:, :, 0:MEM], xt_d.ap()[:, :, 0:MEM])
        wext = consts.tile([128, 4, KEXP], dt.bfloat16)
        nc.sync.dma_start(wext[:], wext_d.ap())
        trign = consts.tile([128, SEQ], dt.float32)
        nc.sync.dma_start(trign[:], trign_d.ap())
        gidx = consts.tile([128, 16], dt.int16)
        nc.sync.dma_start(gidx[:], gidx_d.ap())
        gmat = consts.tile([128, TOT], dt.bfloat16)
        nc.sync.dma_start(gmat[:], gmat_d.ap())
        mask01 = consts.tile([128, 2, TOT], dt.bfloat16)
        nc.sync.dma_start(mask01[:], mask_d.ap())
        wout = consts.tile([128, 4, 512], dt.bfloat16)
        nc.sync.dma_start(wout[:], wout_d.ap())
        bout = consts.tile([128, 512], dt.float32)
        nc.sync.dma_start(bout[:], bout_d.ap())

        # persistent intermediates
        qhatT = consts.tile([128, 4, SEQ], dt.bfloat16)  # (q+u)^T  [hd, n]
        qtldT = consts.tile([128, 4, SEQ], dt.bfloat16)  # (q+v)^T  [hd, n]
        kT = consts.tile([128, 4, TOT], dt.bfloat16)     # k^T      [hd, m]
        vv = consts.tile([128, 8, 512], dt.bfloat16)     # V        [m, hd]
        avt = consts.tile([128, 4, SEQ], dt.bfloat16)    # attnV^T  [hd, n]
        ubt = consts.tile([128, 8, SEQ], dt.bfloat16)    # u^T per head [44, n]
                                                         # at the head's base

        with (
            tc.tile_pool(name="mps", bufs=3, space="PSUM") as mps,
            tc.tile_pool(name="pvps", bufs=2, space="PSUM") as pvps,
            tc.tile_pool(name="hsb", bufs=6) as hsb,
            tc.tile_pool(name="eacp", bufs=8) as eacp,
        ):
            # alternate PSUM->SBUF cast copies between ACT and DVE
            _cp = [0]

            def copy_ps(dst, src):
                _cp[0] ^= 1
                if _cp[0]:
                    nc.scalar.copy(dst, src)
                else:
                    nc.vector.tensor_copy(dst, src)

            # q^T per head pair, then add u/v (tensor_scalar: f32 psum in,
            # bf16 out — probe-verified)
            for hp in range(4):
                psw = mps.tile([128, 1024], dt.float32, tag="m")
                ps = psw[:, 0:SEQ]
                for ch in range(4):
                    nc.tensor.matmul(ps, wqkv[:, ch, 128 * hp:128 * (hp + 1)],
                                     xt[:, ch, MEM:TOT],
                                     start=(ch == 0), stop=(ch == 3))
                nc.vector.tensor_scalar_add(qhatT[:, hp, :], ps, u2[:])
                nc.vector.tensor_scalar_add(qtldT[:, hp, :], ps, v2[:])

            # u^T per head: z = Wext_h^T @ qtld_h (K=64 -> 44), then the
            # elementwise n-trig rotation, cast to bf16.  All APs at the
            # head's partition base pb (consistent PE tile row).
            def emit_u(h):
                hp, par = h // 2, h % 2
                pb = 64 * par
                psw = mps.tile([128, 1024], dt.float32, tag="m")
                zs = psw[pb:pb + KEXP, 0:SEQ]
                nc.tensor.matmul(zs, wext[pb:pb + 64, hp, :],
                                 qtldT[pb:pb + 64, hp, :],
                                 start=True, stop=True)
                uf = hsb.tile([128, SEQ], dt.float32, tag="uf")
                nc.vector.tensor_mul(uf[pb:pb + KEXP, :], zs,
                                     trign[pb:pb + KEXP, :])
                copy_ps(ubt[pb:pb + KEXP, h, :], uf[pb:pb + KEXP, :])

            # k^T per head pair
            def emit_kt(kt_hps):
              for hp in kt_hps:
                ps = mps.tile([128, 1024], dt.float32, tag="m")
                for mh in range(2):
                    for ch in range(4):
                        nc.tensor.matmul(
                            ps[:, 512 * mh:512 * (mh + 1)],
                            wqkv[:, ch, 512 + 128 * hp:512 + 128 * (hp + 1)],
                            xt[:, ch, 512 * mh:512 * (mh + 1)],
                            start=(ch == 0), stop=(ch == 3))
                copy_ps(kT[:, hp, :], ps[:])

            # V in [m, hd] layout, two m-chunks per PSUM tile
            def emit_v(v_mc0s):
              for mc0 in v_mc0s:
                ps = mps.tile([128, 1024], dt.float32, tag="m")
                for k2 in range(2):
                    mc = mc0 + k2
                    for ch in range(4):
                        nc.tensor.matmul(
                            ps[:, 512 * k2:512 * (k2 + 1)],
                            xt[:, ch, 128 * mc:128 * (mc + 1)],
                            wqkv[:, ch, 1024:1536],
                            start=(ch == 0), stop=(ch == 3))
                copy_ps(vv[:, mc0:mc0 + 2, :], ps[:])

            lib_inst = nc.gpsimd.load_library(library_config.mlp)
            attns, attnTs = {}, {}
            _nrm = [0]

            # scores = ac (K=64) + bd (K=44) accumulated in PSUM; one exp;
            # fused mask-product+rowsum (mixed STT); normalize; gather.
            def attn_head(h):
                hp, par = h // 2, h % 2
                pb = 64 * par
                attn = hsb.tile([128, 2, TOT], dt.bfloat16, tag="attn")
                attns[h] = attn
                for n2 in range(2):
                    ps = mps.tile([128, 1024], dt.float32, tag="m")
                    for mh in range(2):
                        nc.tensor.matmul(
                            ps[:, 512 * mh:512 * (mh + 1)],
                            qhatT[pb:pb + 64, hp, 128 * n2:128 * (n2 + 1)],
                            kT[pb:pb + 64, hp, 512 * mh:512 * (mh + 1)],
                            start=True, stop=False)
                    for mh in range(2):
                        nc.tensor.matmul(
                            ps[:, 512 * mh:512 * (mh + 1)],
                            ubt[pb:pb + KEXP, h, 128 * n2:128 * (n2 + 1)],
                            gmat[pb:pb + KEXP, 512 * mh:512 * (mh + 1)],
                            start=False, stop=True)
                    eac = eacp.tile([128, TOT], dt.float32, tag="eac")
                    nc.scalar.activation(eac[:], ps[:], func=AF.Exp,
                                         scale=SCALE)
                    rs = hsb.tile([128, 1], dt.float32, tag="rs")
                    nc.vector.scalar_tensor_tensor(
                        out=eac[:], in0=eac[:], scalar=1.0,
                        in1=mask01[:, n2, :], op0=ALU.mult, op1=ALU.mult,
                        accum_out=rs[:])
                    rec = hsb.tile([128, 1], dt.float32, tag="rec")
                    nc.vector.reciprocal(rec[:], rs[:])
                    _nrm[0] ^= 1
                    if _nrm[0]:
                        nc.scalar.mul(attn[:, n2, :], eac[:], rec[:])
                    else:
                        nc.vector.tensor_scalar_mul(attn[:, n2, :], eac[:],
                                                    rec[:])
                attnT = hsb.tile([128, 8, SEQ], dt.bfloat16, tag="attnT")
                attnTs[h] = attnT
                gth = nc.gpsimd.dma_gather(
                    out_ap=attnT[:], in_ap=attn[:], idxs_ap=gidx[:],
                    num_idxs=SEQ, num_idxs_reg=SEQ, elem_size=TOT,
                    transpose=True, sbuf_tokens_per_rank=128,
                    sbuf_free_dim_per_rank=2 * TOT,
                    sbuf_free_dim_pad_per_rank=0, sbuf_byte_offset=0)
                add_dep_helper(gth.ins, lib_inst.ins,
                               reason="dma_gather needs mlp gpsimd library")

            def pv_pair(g):                  # PV for head pair (col-tiled)
                pvt = pvps.tile([128, SEQ], dt.float32, tag="pv")
                for par in range(2):
                    h = 2 * g + par
                    pb = 64 * par
                    for mc in range(8):
                        nc.tensor.matmul(
                            pvt[pb:pb + 64, :],
                            vv[:, mc, 64 * h:64 * (h + 1)],
                            attnTs[h][:, mc, :],
                            start=(mc == 0), stop=(mc == 7),
                            tile_position=(0, pb))
                    nc.vector.tensor_copy(avt[pb:pb + 64, g, :],
                                          pvt[pb:pb + 64, :])

            # ---- schedule: one continuous PE stream, attention heads
            # interleaved with the remaining projections; PVs trail their
            # gathers.
            for h in range(8):
                emit_u(h)
            emit_kt((0, 1))
            attn_head(0)
            attn_head(1)
            attn_head(2)
            attn_head(3)
            emit_kt((2, 3))
            emit_v((0, 2))
            emit_v((4, 6))
            pv_pair(0)
            attn_head(4)
            attn_head(5)
            pv_pair(1)
            attn_head(6)
            attn_head(7)
            pv_pair(2)
            pv_pair(3)

            # ---------------- phase 3: output projection
            for n2 in range(2):
                psw = mps.tile([128, 1024], dt.float32, tag="m")
                ps = psw[:, 0:512]
                for c4 in range(4):
                    nc.tensor.matmul(ps,
                                     avt[:, c4, 128 * n2:128 * (n2 + 1)],
                                     wout[:, c4, :],
                                     start=(c4 == 0), stop=(c4 == 3))
                osb = hsb.tile([128, 512], dt.float32, tag="osb")
                nc.vector.tensor_add(osb[:], ps, bout[:])
                nc.sync.dma_start(out_d.ap()[128 * n2:128 * (n2 + 1), :], osb[:])


# ---------------------------------------------------------------- host wrapper
_PROGRAM = None


def _get_program():
    global _PROGRAM
    if _PROGRAM is None:
        _PROGRAM = build_program()
    return _PROGRAM


def _chunked(w, nchunk):
    """(128*nchunk, F) -> (128, nchunk, F) with [p, c, f] = w[128c + p, f]."""
    f = w.shape[1]
    return np.ascontiguousarray(w.reshape(nchunk, 128, f).transpose(1, 0, 2))


def make_in_maps(x, memory, W_qkv, W_rel, W_out, b_out, u_emb, v_emb):
    x = np.asarray(x, dtype=F32)
    memory = np.asarray(memory, dtype=F32)
    W_qkv = np.asarray(W_qkv, dtype=F32)
    W_rel = np.asarray(W_rel, dtype=F32)
    W_out = np.asarray(W_out, dtype=F32)
    b_out = np.asarray(b_out, dtype=F32)
    u_emb = np.asarray(u_emb, dtype=F32)
    v_emb = np.asarray(v_emb, dtype=F32)

    # sinusoid tables for the rank-44 bd expansion (f64 for exactness)
    omg = (2.0 ** np.arange(1 - NOCT, 1).astype(np.float64)) * np.pi
    n = np.arange(SEQ, dtype=np.float64)
    m = np.arange(TOT, dtype=np.float64)
    cosN = np.cos(omg[:, None] * n[None, :])
    sinN = np.sin(omg[:, None] * n[None, :])
    sinM = np.sin(omg[:, None] * (MEM - m)[None, :])
    cosM = np.cos(omg[:, None] * (MEM - m)[None, :])
    trign44 = np.concatenate([cosN, sinN, cosN, sinN], 0).astype(F32)  # (44,256)
    gmat44 = np.concatenate([sinM, cosM, cosM, sinM], 0).astype(F32)   # (44,1024)
    # duplicate at partition base 64 so odd heads' matmuls stay in their
    # PE tile row
    trign = np.zeros((128, SEQ), F32)
    trign[0:KEXP] = trign44
    trign[64:64 + KEXP] = trign44
    gmat = np.zeros((128, TOT), F32)
    gmat[0:KEXP] = gmat44
    gmat[64:64 + KEXP] = gmat44
    gmat = gmat.astype(BF16)

    # Wext per head: [Ws, Ws, Wc, -Wc] columns, laid out to match qtldT
    Ws = W_rel[0:NOCT].reshape(NOCT, NHEAD, DHEAD)    # (o, h, d)
    Wc = W_rel[NOCT:2 * NOCT].reshape(NOCT, NHEAD, DHEAD)
    wext = np.zeros((128, 4, KEXP), F32)
    for h in range(NHEAD):
        hp, par = h // 2, h % 2
        pb = 64 * par
        we = np.concatenate([Ws[:, h, :], Ws[:, h, :],
                             Wc[:, h, :], -Wc[:, h, :]], 0).T  # (64, 44)
        wext[pb:pb + 64, hp, :] = we
    wext = wext.astype(BF16)

    # causal 0/1 mask, per n2 slab
    nn = np.arange(SEQ)[:, None]
    mask = (np.arange(TOT)[None, :] <= MEM + nn).astype(F32)  # (256, 1024)
    mask01 = np.stack([mask[0:128], mask[128:256]], 1).astype(BF16)

    wqkv = _chunked(W_qkv, 4).astype(BF16)           # (128, 4, 1536)
    wout = _chunked(W_out, 4).astype(BF16)           # (128, 4, 512)
    bout = np.tile(b_out[None, :], (128, 1)).astype(F32)
    u2 = np.tile(u_emb, 2)[:, None].astype(F32)
    v2 = np.tile(v_emb, 2)[:, None].astype(F32)
    p = np.arange(128)[:, None] % 16
    s = np.arange(16)[None, :]
    gidx = (s * 16 + p).astype(np.int16)             # (128, 16)

    shared = dict(wqkv=wqkv, wext=wext, trign=trign, gmat=gmat,
                  mask01=mask01, wout=wout, bout=bout, u2=u2, v2=v2,
                  gidx=gidx)
    in_maps = []
    for c in range(B):
        X = np.concatenate([memory[c], x[c]], axis=0)          # (1024, 512)
        xt = _chunked(np.ascontiguousarray(X.T), 4).astype(BF16)  # (128,4,1024)
        in_maps.append(dict(xt=xt, **shared))
    return in_maps


def run(in_maps, trace=False, **kw):
    nc = _get_program()
    res = run_bass_kernel_spmd(nc, in_maps, core_ids=list(range(B)),
                               trace=trace, **kw)
    out = np.stack([res.results[c]["out"] for c in range(B)]).astype(F32)
    return out, res


def kernel(x, memory, W_qkv, W_rel, W_out, b_out, u_emb, v_emb):
    in_maps = make_in_maps(x, memory, W_qkv, W_rel, W_out, b_out, u_emb, v_emb)
    out, _ = run(in_maps)
    return out.reshape(B, SEQ, DIM)


# revision 56
# speedup vs baseline: 1.1363x; 1.1363x over previous
"""Trainium2 Bass kernel for nn_Attention_386547057357 (Transformer-XL style
relative-position sparse attention).

Sharding: data-parallel over batch — core c computes batch element c.
All weights replicated per core.

Math (per batch element):
    X = [memory; x]  (1024, 512)
    q = x @ W_q  (256, 512);  k = X @ W_k;  v = X @ W_v
    qhat = q + u_emb (per head);  qtld = q + v_emb
    ac[n, m]  = qhat_h[n] . k_h[m]                 (= term_a + term_c)
    bd[n, m]  = qtld_h[n] . RW[768 + n - m]        (= term_b + term_d)
    scores    = (ac + bd) * scale  + causal mask (m <= 768 + n)
    out = softmax(scores) @ v @ W_out + b_out

Key trick (rank-44 shift elimination): RW = R @ W_rel has rank <= 22 and
R's rows are sinusoids, so by the angle-addition identities

    bd_h[n, m] = sum_o  z_s,o[n] sin(w_o (768 + n - m))
                      + z_c,o[n] cos(w_o (768 + n - m))
               = sum_{k<44} u_k[n] * G_k[m]

with z_h = qtld_h @ W_rel_h^T (a K=64 -> 44 matmul),
u = [z_s*cosN, z_s*sinN, z_c*cosN, -z_c*sinN] (elementwise n-trig products)
and G = [sinM, cosM, cosM, sinM] host constants, sinM_o[m]=sin(w_o(768-m)).
So the relative-shift term is a plain K=44 matmul accumulated onto ac in
PSUM — no DRAM round trip, no shift at all.  The causal mask is a 0/1
bf16 host tile applied by the fused product+rowsum (scalar_tensor_tensor)
that also computes the softmax denominator.  Softmax skips max-subtraction
(logits are small; fp32 exp).  attn transpose for PV via gpsimd dma_gather.
All PE operands bf16; PSUM accumulation fp32; output fp32.

HW pitfall encoded here: all matmuls of one PSUM accumulation group must
share the lhsT/rhs base partition (PE tile row) — mixing base 0 and 64 in
one group hard-faults the device.  Hence the dual-offset copies of the
trig/G tables and per-head u rows at the head's own partition base.
"""

import sys

sys.path.insert(0, "/opt/trn_rl_repo")

import numpy as np
import ml_dtypes

import concourse.bass as bass
import concourse.mybir as mybir
import concourse.tile as tile
from concourse import bacc, library_config
from concourse.bass_utils import run_bass_kernel_spmd
from concourse.tile_rust import add_dep_helper

BF16 = ml_dtypes.bfloat16
F32 = np.float32

DIM = 512
NHEAD = 8
DHEAD = 64
CTX = 1024
NOCT = 11
B = 8
SEQ = 256
MEM = 768
TOT = MEM + SEQ  # 1024
SCALE = DHEAD ** -0.5  # 0.125
KEXP = 4 * NOCT  # 44: rank of the bd expansion

dt = mybir.dt
AF = mybir.ActivationFunctionType
ALU = mybir.AluOpType


# ---------------------------------------------------------------- bass program
def build_program():
    nc = bacc.Bacc("TRN2", target_bir_lowering=False, debug=False)

    xt_d = nc.dram_tensor("xt", [128, 4, TOT], dt.bfloat16, kind="ExternalInput")
    wqkv_d = nc.dram_tensor("wqkv", [128, 4, 1536], dt.bfloat16, kind="ExternalInput")
    wext_d = nc.dram_tensor("wext", [128, 4, KEXP], dt.bfloat16, kind="ExternalInput")
    trign_d = nc.dram_tensor("trign", [128, SEQ], dt.float32, kind="ExternalInput")
    gmat_d = nc.dram_tensor("gmat", [128, TOT], dt.bfloat16, kind="ExternalInput")
    mask_d = nc.dram_tensor("mask01", [128, 2, TOT], dt.bfloat16, kind="ExternalInput")
    wout_d = nc.dram_tensor("wout", [128, 4, 512], dt.bfloat16, kind="ExternalInput")
    bout_d = nc.dram_tensor("bout", [128, 512], dt.float32, kind="ExternalInput")
    u2_d = nc.dram_tensor("u2", [128, 1], dt.float32, kind="ExternalInput")
    v2_d = nc.dram_tensor("v2", [128, 1], dt.float32, kind="ExternalInput")
    gidx_d = nc.dram_tensor("gidx", [128, 16], dt.int16, kind="ExternalInput")
    out_d = nc.dram_tensor("out", [SEQ, 512], dt.float32, kind="ExternalOutput")

    with tile.TileContext(nc) as tc:
        _body(tc, xt_d, wqkv_d, wext_d, trign_d, gmat_d, mask_d, wout_d,
              bout_d, u2_d, v2_d, gidx_d, out_d)
    nc.compile()
    return nc


def _body(tc, xt_d, wqkv_d, wext_d, trign_d, gmat_d, mask_d, wout_d, bout_d,
          u2_d, v2_d, gidx_d, out_d):
    nc = tc.nc
    from contextlib import ExitStack

    with ExitStack() as ctx:
        consts = ctx.enter_context(tc.tile_pool(name="consts", bufs=1))

        # ---- loads.  The q projection only needs the x-part of X^T
        # (cols MEM:) and the q-columns of W_qkv — load those first so the
        # PE starts ~5us earlier; the k/v parts stream in behind.
        xt = consts.tile([128, 4, TOT], dt.bfloat16)
        wqkv = consts.tile([128, 4, 1536], dt.bfloat16)
        nc.sync.dma_start(xt[:, :, MEM:TOT], xt_d.ap()[:, :, MEM:TOT])
        nc.sync.dma_start(wqkv[:, :, 0:512], wqkv_d.ap()[:, :, 0:512])
        u2 = consts.tile([128, 1], dt.float32)
        nc.sync.dma_start(u2[:], u2_d.ap())
        v2 = consts.tile([128, 1], dt.float32)
        nc.sync.dma_start(v2[:], v2_d.ap())
        nc.sync.dma_start(wqkv[:, :, 512:1536], wqkv_d.ap()[:, :, 512:1536])
        nc.sync.dma_start(xt[:, :, 0:MEM], xt_d.ap()[:, :, 0:MEM])
        wext = consts.tile([128, 4, KEXP], dt.bfloat16)
        nc.sync.dma_start(wext[:], wext_d.ap())
        trign = consts.tile([128, SEQ], dt.float32)
        nc.sync.dma_start(trign[:], trign_d.ap())
        gidx = consts.tile([128, 16], dt.int16)
        nc.sync.dma_start(gidx[:], gidx_d.ap())
        gmat = consts.tile([128, TOT], dt.bfloat16)
        nc.sync.dma_start(gmat[:], gmat_d.ap())
        mask01 = consts.tile([128, 2, TOT], dt.bfloat16)
        nc.sync.dma_start(mask01[:], mask_d.ap())
        wout = consts.tile([128, 4, 512], dt.bfloat16)
        nc.sync.dma_start(wout[:], wout_d.ap())
        bout = consts.tile([128, 512], dt.float32)
        nc.sync.dma_start(bout[:], bout_d.ap())

        # persistent intermediates
        qhatT = consts.tile([128, 4, SEQ], dt.bfloat16)  # (q+u)^T  [hd, n]
        qtldT = consts.tile([128, 4, SEQ], dt.bfloat16)  # (q+v)^T  [hd, n]
        kT = consts.tile([128, 4, TOT], dt.bfloat16)     # k^T      [hd, m]
        vv = consts.tile([128, 8, 512], dt.bfloat16)     # V        [m, hd]
        avt = consts.tile([128, 4, SEQ], dt.bfloat16)    # attnV^T  [hd, n]
        ubt = consts.tile([128, 8, SEQ], dt.bfloat16)    # u^T per head [44, n]
                                                         # at the head's base

        with (
            tc.tile_pool(name="mps", bufs=3, space="PSUM") as mps,
            tc.tile_pool(name="pvps", bufs=2, space="PSUM") as pvps,
            tc.tile_pool(name="hsb", bufs=6) as hsb,
            tc.tile_pool(name="eacp", bufs=8) as eacp,
        ):
            # alternate PSUM->SBUF cast copies between ACT and DVE
            _cp = [0]

            def copy_ps(dst, src):
                _cp[0] ^= 1
                if _cp[0]:
                    nc.scalar.copy(dst, src)
                else:
                    nc.vector.tensor_copy(dst, src)

            # q^T per head pair, then add u/v (tensor_scalar: f32 psum in,
            # bf16 out — probe-verified)
            for hp in range(4):
                psw = mps.tile([128, 1024], dt.float32, tag="m")
                ps = psw[:, 0:SEQ]
                for ch in range(4):
                    nc.tensor.matmul(ps, wqkv[:, ch, 128 * hp:128 * (hp + 1)],
                                     xt[:, ch, MEM:TOT],
                                     start=(ch == 0), stop=(ch == 3))
                nc.vector.tensor_scalar_add(qhatT[:, hp, :], ps, u2[:])
                nc.vector.tensor_scalar_add(qtldT[:, hp, :], ps, v2[:])

            # u^T per head: z = Wext_h^T @ qtld_h (K=64 -> 44), then the
            # elementwise n-trig rotation, cast to bf16.  All APs at the
            # head's partition base pb (consistent PE tile row).
            def emit_u(h):
                hp, par = h // 2, h % 2
                pb = 64 * par
                psw = mps.tile([128, 1024], dt.float32, tag="m")
                zs = psw[pb:pb + KEXP, 0:SEQ]
                nc.tensor.matmul(zs, wext[pb:pb + 64, hp, :],
                                 qtldT[pb:pb + 64, hp, :],
                                 start=True, stop=True)
                uf = hsb.tile([128, SEQ], dt.float32, tag="uf")
                nc.vector.tensor_mul(uf[pb:pb + KEXP, :], zs,
                                     trign[pb:pb + KEXP, :])
                copy_ps(ubt[pb:pb + KEXP, h, :], uf[pb:pb + KEXP, :])

            # k^T per head pair
            def emit_kt(kt_hps):
              for hp in kt_hps:
                ps = mps.tile([128, 1024], dt.float32, tag="m")
                for mh in range(2):
                    for ch in range(4):
                        nc.tensor.matmul(
                            ps[:, 512 * mh:512 * (mh + 1)],
                            wqkv[:, ch, 512 + 128 * hp:512 + 128 * (hp + 1)],
                            xt[:, ch, 512 * mh:512 * (mh + 1)],
                            start=(ch == 0), stop=(ch == 3))
                copy_ps(kT[:, hp, :], ps[:])

            # V in [m, hd] layout, two m-chunks per PSUM tile
            def emit_v(v_mc0s):
              for mc0 in v_mc0s:
                ps = mps.tile([128, 1024], dt.float32, tag="m")
                for k2 in range(2):
                    mc = mc0 + k2
                    for ch in range(4):
                        nc.tensor.matmul(
                            ps[:, 512 * k2:512 * (k2 + 1)],
                            xt[:, ch, 128 * mc:128 * (mc + 1)],
                            wqkv[:, ch, 1024:1536],
                            start=(ch == 0), stop=(ch == 3))
                copy_ps(vv[:, mc0:mc0 + 2, :], ps[:])

            lib_inst = nc.gpsimd.load_library(library_config.mlp)
            attns, attnTs = {}, {}
            _nrm = [0]

            # scores = ac (K=64) + bd (K=44) accumulated in PSUM; one exp;
            # fused mask-product+rowsum (mixed STT); normalize; gather.
            def attn_head(h):
                hp, par = h // 2, h % 2
                pb = 64 * par
                attn = hsb.tile([128, 2, TOT], dt.bfloat16, tag="attn")
                attns[h] = attn
                for n2 in range(2):
                    ps = mps.tile([128, 1024], dt.float32, tag="m")
                    for mh in range(2):
                        nc.tensor.matmul(
                            ps[:, 512 * mh:512 * (mh + 1)],
                            qhatT[pb:pb + 64, hp, 128 * n2:128 * (n2 + 1)],
                            kT[pb:pb + 64, hp, 512 * mh:512 * (mh + 1)],
                            start=True, stop=False)
                    for mh in range(2):
                        nc.tensor.matmul(
                            ps[:, 512 * mh:512 * (mh + 1)],
                            ubt[pb:pb + KEXP, h, 128 * n2:128 * (n2 + 1)],
                            gmat[pb:pb + KEXP, 512 * mh:512 * (mh + 1)],
                            start=False, stop=True)
                    eac = eacp.tile([128, TOT], dt.float32, tag="eac")
                    nc.scalar.activation(eac[:], ps[:], func=AF.Exp,
                                         scale=SCALE)
                    rs = hsb.tile([128, 1], dt.float32, tag="rs")
                    nc.vector.scalar_tensor_tensor(
                        out=eac[:], in0=eac[:], scalar=1.0,
                        in1=mask01[:, n2, :], op0=ALU.mult, op1=ALU.mult,
                        accum_out=rs[:])
                    rec = hsb.tile([128, 1], dt.float32, tag="rec")
                    nc.vector.reciprocal(rec[:], rs[:])
                    _nrm[0] ^= 1
                    if _nrm[0]:
                        nc.scalar.mul(attn[:, n2, :], eac[:], rec[:])
                    else:
                        nc.vector.tensor_scalar_mul(attn[:, n2, :], eac[:],
                                                    rec[:])
                attnT = hsb.tile([128, 8, SEQ], dt.bfloat16, tag="attnT")
                attnTs[h] = attnT
                gth = nc.gpsimd.dma_gather(
                    out_ap=attnT[:], in_ap=attn[:], idxs_ap=gidx[:],
                    num_idxs=SEQ, num_idxs_reg=SEQ, elem_size=TOT,
                    transpose=True, sbuf_tokens_per_rank=128,
                    sbuf_free_dim_per_rank=2 * TOT,
                    sbuf_free_dim_pad_per_rank=0, sbuf_byte_offset=0)
                add_dep_helper(gth.ins, lib_inst.ins,
                               reason="dma_gather needs mlp gpsimd library")

            def pv_pair(g):                  # PV for head pair (col-tiled)
                pvt = pvps.tile([128, SEQ], dt.float32, tag="pv")
                for par in range(2):
                    h = 2 * g + par
                    pb = 64 * par
                    for mc in range(8):
                        nc.tensor.matmul(
                            pvt[pb:pb + 64, :],
                            vv[:, mc, 64 * h:64 * (h + 1)],
                            attnTs[h][:, mc, :],
                            start=(mc == 0), stop=(mc == 7),
                            tile_position=(0, pb))
                    nc.vector.tensor_copy(avt[pb:pb + 64, g, :],
                                          pvt[pb:pb + 64, :])

            # ---- schedule: one continuous PE stream, attention heads
            # interleaved with the remaining projections; PVs trail their
            # gathers.
            for h in range(8):
                emit_u(h)
            emit_kt((0, 1))
            attn_head(0)
            attn_head(1)
            emit_kt((2,))
            attn_head(2)
            emit_kt((3,))
            attn_head(3)
            emit_v((0, 2))
            emit_v((4, 6))
            pv_pair(0)
            attn_head(4)
            attn_head(5)
            pv_pair(1)
            attn_head(6)
            attn_head(7)
            pv_pair(2)
            pv_pair(3)

            # ---------------- phase 3: output projection
            for n2 in range(2):
                psw = mps.tile([128, 1024], dt.float32, tag="m")
                ps = psw[:, 0:512]
                for c4 in range(4):
                    nc.tensor.matmul(ps,
                                     avt[:, c4, 128 * n2:128 * (n2 + 1)],
                                     wout[:, c4, :],
                                     start=(c4 == 0), stop=(c4 == 3))
                osb = hsb.tile([128, 512], dt.float32, tag="osb")
                nc.vector.tensor_add(osb[:], ps, bout[:])
                nc.sync.dma_start(out_d.ap()[128 * n2:128 * (n2 + 1), :], osb[:])


# ---------------------------------------------------------------- host wrapper
_PROGRAM = None


def _get_program():
    global _PROGRAM
    if _PROGRAM is None:
        _PROGRAM = build_program()
    return _PROGRAM


def _chunked(w, nchunk):
    """(128*nchunk, F) -> (128, nchunk, F) with [p, c, f] = w[128c + p, f]."""
    f = w.shape[1]
    return np.ascontiguousarray(w.reshape(nchunk, 128, f).transpose(1, 0, 2))


def make_in_maps(x, memory, W_qkv, W_rel, W_out, b_out, u_emb, v_emb):
    x = np.asarray(x, dtype=F32)
    memory = np.asarray(memory, dtype=F32)
    W_qkv = np.asarray(W_qkv, dtype=F32)
    W_rel = np.asarray(W_rel, dtype=F32)
    W_out = np.asarray(W_out, dtype=F32)
    b_out = np.asarray(b_out, dtype=F32)
    u_emb = np.asarray(u_emb, dtype=F32)
    v_emb = np.asarray(v_emb, dtype=F32)

    # sinusoid tables for the rank-44 bd expansion (f64 for exactness)
    omg = (2.0 ** np.arange(1 - NOCT, 1).astype(np.float64)) * np.pi
    n = np.arange(SEQ, dtype=np.float64)
    m = np.arange(TOT, dtype=np.float64)
    cosN = np.cos(omg[:, None] * n[None, :])
    sinN = np.sin(omg[:, None] * n[None, :])
    sinM = np.sin(omg[:, None] * (MEM - m)[None, :])
    cosM = np.cos(omg[:, None] * (MEM - m)[None, :])
    trign44 = np.concatenate([cosN, sinN, cosN, sinN], 0).astype(F32)  # (44,256)
    gmat44 = np.concatenate([sinM, cosM, cosM, sinM], 0).astype(F32)   # (44,1024)
    # duplicate at partition base 64 so odd heads' matmuls stay in their
    # PE tile row
    trign = np.zeros((128, SEQ), F32)
    trign[0:KEXP] = trign44
    trign[64:64 + KEXP] = trign44
    gmat = np.zeros((128, TOT), F32)
    gmat[0:KEXP] = gmat44
    gmat[64:64 + KEXP] = gmat44
    gmat = gmat.astype(BF16)

    # Wext per head: [Ws, Ws, Wc, -Wc] columns, laid out to match qtldT
    Ws = W_rel[0:NOCT].reshape(NOCT, NHEAD, DHEAD)    # (o, h, d)
    Wc = W_rel[NOCT:2 * NOCT].reshape(NOCT, NHEAD, DHEAD)
    wext = np.zeros((128, 4, KEXP), F32)
    for h in range(NHEAD):
        hp, par = h // 2, h % 2
        pb = 64 * par
        we = np.concatenate([Ws[:, h, :], Ws[:, h, :],
                             Wc[:, h, :], -Wc[:, h, :]], 0).T  # (64, 44)
        wext[pb:pb + 64, hp, :] = we
    wext = wext.astype(BF16)

    # causal 0/1 mask, per n2 slab
    nn = np.arange(SEQ)[:, None]
    mask = (np.arange(TOT)[None, :] <= MEM + nn).astype(F32)  # (256, 1024)
    mask01 = np.stack([mask[0:128], mask[128:256]], 1).astype(BF16)

    wqkv = _chunked(W_qkv, 4).astype(BF16)           # (128, 4, 1536)
    wout = _chunked(W_out, 4).astype(BF16)           # (128, 4, 512)
    bout = np.tile(b_out[None, :], (128, 1)).astype(F32)
    u2 = np.tile(u_emb, 2)[:, None].astype(F32)
    v2 = np.tile(v_emb, 2)[:, None].astype(F32)
    p = np.arange(128)[:, None] % 16
    s = np.arange(16)[None, :]
    gidx = (s * 16 + p).astype(np.int16)             # (128, 16)

    shared = dict(wqkv=wqkv, wext=wext, trign=trign, gmat=gmat,
                  mask01=mask01, wout=wout, bout=bout, u2=u2, v2=v2,
                  gidx=gidx)
    in_maps = []
    for c in range(B):
        X = np.concatenate([memory[c], x[c]], axis=0)          # (1024, 512)
        xt = _chunked(np.ascontiguousarray(X.T), 4).astype(BF16)  # (128,4,1024)
        in_maps.append(dict(xt=xt, **shared))
    return in_maps


def run(in_maps, trace=False, **kw):
    nc = _get_program()
    res = run_bass_kernel_spmd(nc, in_maps, core_ids=list(range(B)),
                               trace=trace, **kw)
    out = np.stack([res.results[c]["out"] for c in range(B)]).astype(F32)
    return out, res


def kernel(x, memory, W_qkv, W_rel, W_out, b_out, u_emb, v_emb):
    in_maps = make_in_maps(x, memory, W_qkv, W_rel, W_out, b_out, u_emb, v_emb)
    out, _ = run(in_maps)
    return out.reshape(B, SEQ, DIM)
